# revision 12
# baseline (speedup 1.0000x reference)
"""Trainium2 Bass kernel for nn_AttentionModulator.

Reference computation (per full input):
    x = attn_weights + noise * 0.1
    hyper = isin(input_ids, hyperfocus_ids)          # [B, K]
    avoid = isin(input_ids, avoid_ids)               # [B, K]
    scale = where(hyper, 1.18, 1.0) * where(avoid, 0.999, 1.0)
    out = softmax(x * scale[:, None, None, :], axis=-1)

Shapes: attn/noise [B=2, H=16, Q=1024, K=2048] f32, input_ids [B, K] i64,
hyperfocus_ids/avoid_ids [64] i64.  Output [B, H, Q, K] f32.

Sharding: flatten (B, H) -> 32 slices, 4 contiguous slices per core across
8 cores (cores 0-3 get b=0, cores 4-7 get b=1, so each core needs a single
batch row of input_ids).  Token-id sets are replicated.  All compute is
local per (b, h) slice; no collectives.
"""

import numpy as np

import concourse.bass as bass
import concourse.tile as tile
from concourse import bacc, mybir
from concourse.bass_utils import run_bass_kernel_spmd

F32 = mybir.dt.float32
OP = mybir.AluOpType
AFT = mybir.ActivationFunctionType

N_CORES = 8
B, H, Q, K = 2, 16, 1024, 2048
NSET = 64
SLICES_PER_CORE = (B * H) // N_CORES  # 4
P = 128  # partitions / q rows per tile

DISTRACTION_LEVEL = 0.1
# match reference: 1.0 + 1.8*0.1 and 1.0 - 0.01*0.1 evaluated in f64 then
# rounded to f32 by jax
HYPER_DELTA = float(1.0 + 1.8 * 0.1) - 1.0    # 0.18000000000000016
AVOID_DELTA = float(1.0 - 0.01 * 0.1) - 1.0   # -0.0009999999999999454


def build_nc(
    slices=SLICES_PER_CORE, q=Q, k=K, bufs=4, reps=1, qb=1, store_eng="sync"
):
    """Build the per-core SPMD Bass module.

    Per-core inputs: attn/noise [slices, q, k] f32, ids [k] f32 (token ids of
    this core's batch row, cast to f32 -- exact for ids < 2^24), hyper/avoid
    [NSET] f32.  Output: out [slices, q, k] f32.
    """
    assert k % P == 0 and q % P == 0
    F = k // P  # ids per partition when k ids are spread over P partitions

    nc = bacc.Bacc("TRN2", target_bir_lowering=False, debug=False)
    attn = nc.dram_tensor("attn", [slices, q, k], F32, kind="ExternalInput").ap()
    noise = nc.dram_tensor("noise", [slices, q, k], F32, kind="ExternalInput").ap()
    ids = nc.dram_tensor("ids", [k], F32, kind="ExternalInput").ap()
    hyper = nc.dram_tensor("hyper", [NSET], F32, kind="ExternalInput").ap()
    avoid = nc.dram_tensor("avoid", [NSET], F32, kind="ExternalInput").ap()
    out = nc.dram_tensor("out", [slices, q, k], F32, kind="ExternalOutput").ap()
    scratch = nc.dram_tensor("scale_scratch", [k], F32).ap()

    with tile.TileContext(nc) as tc:
        with (
            tc.tile_pool(name="setup", bufs=1) as setup_pool,
            tc.tile_pool(name="scale", bufs=1) as scale_pool,
            tc.tile_pool(name="attn", bufs=bufs) as attn_pool,
            tc.tile_pool(name="noise", bufs=bufs) as noise_pool,
            tc.tile_pool(name="stats", bufs=2 * bufs) as stats_pool,
        ):
            # ---- one-time: scale row --------------------------------------
            # ids laid out [P, F] (id index = p*F + f); sets broadcast [P, 64]
            ids_sb = setup_pool.tile([P, F], F32, tag="ids")
            nc.sync.dma_start(ids_sb[:], ids.rearrange("(p f) -> p f", p=P))
            hyper_sb = setup_pool.tile([P, NSET], F32, tag="hyp")
            nc.sync.dma_start(
                hyper_sb[:], hyper.unsqueeze(0).to_broadcast((P, NSET))
            )
            avoid_sb = setup_pool.tile([P, NSET], F32, tag="avd")
            nc.sync.dma_start(
                avoid_sb[:], avoid.unsqueeze(0).to_broadcast((P, NSET))
            )

            # membership: eq[p, f, j] = (ids[p, f] == set[j]); reduce over j
            ids_b = ids_sb[:].unsqueeze(2).to_broadcast((P, F, NSET))
            eq = setup_pool.tile([P, F, NSET], F32, tag="eq")
            hmem = setup_pool.tile([P, F], F32, tag="hmem")
            nc.vector.tensor_tensor(
                eq[:], ids_b, hyper_sb[:].unsqueeze(1).to_broadcast((P, F, NSET)),
                op=OP.is_equal,
            )
            nc.vector.reduce_max(hmem[:], eq[:], axis=mybir.AxisListType.X)
            eq2 = setup_pool.tile([P, F, NSET], F32, tag="eq2")
            amem = setup_pool.tile([P, F], F32, tag="amem")
            nc.vector.tensor_tensor(
                eq2[:], ids_b, avoid_sb[:].unsqueeze(1).to_broadcast((P, F, NSET)),
                op=OP.is_equal,
            )
            nc.vector.reduce_max(amem[:], eq2[:], axis=mybir.AxisListType.X)

            # scale = (1 + 0.18*h) * (1 - 0.001*a)
            nc.vector.tensor_scalar(
                hmem[:], hmem[:], HYPER_DELTA, 1.0, OP.mult, OP.add
            )
            nc.vector.tensor_scalar(
                amem[:], amem[:], AVOID_DELTA, 1.0, OP.mult, OP.add
            )
            nc.vector.tensor_tensor(hmem[:], hmem[:], amem[:], op=OP.mult)

            # bounce through DRAM to broadcast the scale row to all partitions
            nc.sync.dma_start(scratch.rearrange("(p f) -> p f", p=P), hmem[:])
            scale_bc = scale_pool.tile([P, k], F32, tag="scale_bc")
            nc.sync.dma_start(
                scale_bc[:], scratch.unsqueeze(0).to_broadcast((P, k))
            )

            # ---- main loop: softmax((attn + 0.1*noise) * scale) over k ----
            # Values are ~N(0, 1.18) so exp never overflows in f32; skip the
            # max-subtraction pass (matches jax softmax to ~1e-7 rel).
            # qb query-blocks of 128 rows per tile: tiles are [P, qb, k]
            # (qb*k free elements), DMAs move qb MB at once.  Row r of
            # query-block g lives at tile[:, g, :] and softmax reduces per
            # (row, g) over k, so exp/mul run per-g on sub-APs.
            scale_bc3 = scale_bc[:].unsqueeze(1).to_broadcast((P, qb, k))
            store = getattr(nc, store_eng)

            def main_body():
                for s in range(slices):
                    for j in range(q // (P * qb)):
                        rows = slice(j * P * qb, (j + 1) * P * qb)
                        # [qb*P, k] DRAM region viewed as [P, qb, k]
                        a_src = attn[s, rows, :].rearrange(
                            "(g p) k -> p g k", p=P
                        )
                        n_src = noise[s, rows, :].rearrange(
                            "(g p) k -> p g k", p=P
                        )
                        o_dst = out[s, rows, :].rearrange(
                            "(g p) k -> p g k", p=P
                        )
                        a_t = attn_pool.tile([P, qb, k], F32, tag="a")
                        nc.sync.dma_start(a_t[:], a_src)
                        n_t = noise_pool.tile([P, qb, k], F32, tag="n")
                        nc.sync.dma_start(n_t[:], n_src)

                        # n = (noise * 0.1) + attn
                        nc.vector.scalar_tensor_tensor(
                            n_t[:], n_t[:], DISTRACTION_LEVEL, a_t[:],
                            op0=OP.mult, op1=OP.add,
                        )
                        # n *= scale[k]
                        nc.vector.tensor_tensor(
                            n_t[:], n_t[:], scale_bc3, op=OP.mult
                        )
                        # a = exp(n); ssum = rowsum(exp(n)) per query-block
                        ssum = stats_pool.tile([P, qb], F32, tag="ssum")
                        for g in range(qb):
                            nc.scalar.activation(
                                a_t[:, g, :], n_t[:, g, :], AFT.Exp,
                                accum_out=ssum[:, g : g + 1],
                            )
                        rcp = stats_pool.tile([P, qb], F32, tag="rcp")
                        nc.vector.reciprocal(rcp[:], ssum[:])
                        # n = a * (1/ssum)
                        for g in range(qb):
                            nc.scalar.mul(
                                n_t[:, g, :], a_t[:, g, :], rcp[:, g : g + 1]
                            )
                        store.dma_start(o_dst, n_t[:])

            if reps == 1:
                main_body()
            else:
                # benchmarking only: repeat the identical body on a HW loop
                with tc.For_i(0, reps, 1):
                    main_body()

    nc.compile()
    return nc


_NC_CACHE = {}


def _get_nc(reps=1):
    key = (SLICES_PER_CORE, Q, K, reps)
    if key not in _NC_CACHE:
        _NC_CACHE[key] = build_nc(reps=reps)
    return _NC_CACHE[key]


def _shard(attn_weights, noise, input_ids, hyperfocus_ids, avoid_ids):
    attn_flat = np.ascontiguousarray(attn_weights, dtype=np.float32).reshape(
        B * H, Q, K
    )
    noise_flat = np.ascontiguousarray(noise, dtype=np.float32).reshape(B * H, Q, K)
    hyper_f = np.asarray(hyperfocus_ids).astype(np.float32)
    avoid_f = np.asarray(avoid_ids).astype(np.float32)
    ids_f = np.asarray(input_ids).astype(np.float32)  # [B, K]

    in_maps = []
    for c in range(N_CORES):
        lo = c * SLICES_PER_CORE
        b = lo // H
        in_maps.append(
            {
                "attn": attn_flat[lo : lo + SLICES_PER_CORE],
                "noise": noise_flat[lo : lo + SLICES_PER_CORE],
                "ids": ids_f[b],
                "hyper": hyper_f,
                "avoid": avoid_f,
            }
        )
    return in_maps


def run_sharded(in_maps, trace=False, **kwargs):
    nc = _get_nc()
    return run_bass_kernel_spmd(
        nc, in_maps, core_ids=list(range(N_CORES)), trace=trace, **kwargs
    )


def kernel(attn_weights, noise, input_ids, hyperfocus_ids, avoid_ids):
    in_maps = _shard(attn_weights, noise, input_ids, hyperfocus_ids, avoid_ids)
    res = run_sharded(in_maps)
    parts = [res.results[c]["out"] for c in range(N_CORES)]
    full = np.concatenate(parts, axis=0).reshape(B, H, Q, K)
    return full


# revision 16
# speedup vs baseline: 1.1761x; 1.1761x over previous
"""Trainium2 Bass kernel for nn_AttentionModulator.

Reference computation (per full input):
    x = attn_weights + noise * 0.1
    hyper = isin(input_ids, hyperfocus_ids)          # [B, K]
    avoid = isin(input_ids, avoid_ids)               # [B, K]
    scale = where(hyper, 1.18, 1.0) * where(avoid, 0.999, 1.0)
    out = softmax(x * scale[:, None, None, :], axis=-1)

Shapes: attn/noise [B=2, H=16, Q=1024, K=2048] f32, input_ids [B, K] i64,
hyperfocus_ids/avoid_ids [64] i64.  Output [B, H, Q, K] f32.

Sharding: flatten (B, H) -> 32 slices, 4 contiguous slices per core across
8 cores (cores 0-3 get b=0, cores 4-7 get b=1, so each core needs a single
batch row of input_ids).  Token-id sets are replicated.  All compute is
local per (b, h) slice; no collectives.
"""

import numpy as np

import concourse.tile as tile
from concourse import bacc, mybir
from concourse.bass_utils import run_bass_kernel_spmd

F32 = mybir.dt.float32
OP = mybir.AluOpType
AFT = mybir.ActivationFunctionType

N_CORES = 8
B, H, Q, K = 2, 16, 1024, 2048
NSET = 64
SLICES_PER_CORE = (B * H) // N_CORES  # 4
P = 128  # partitions / q rows per tile

DISTRACTION_LEVEL = 0.1
# match reference: 1.0 + 1.8*0.1 and 1.0 - 0.01*0.1 evaluated in f64 then
# rounded to f32 by jax
HYPER_DELTA = float(1.0 + 1.8 * 0.1) - 1.0    # 0.18000000000000016
AVOID_DELTA = float(1.0 - 0.01 * 0.1) - 1.0   # -0.0009999999999999454


def build_nc(
    slices=SLICES_PER_CORE, q=Q, k=K, bufs=4, reps=1, qb=1, store_eng="sync",
    dma_only=False,
):
    """Build the per-core SPMD Bass module.

    Per-core inputs: attn/noise [slices, q, k] f32, ids [k] f32 (token ids of
    this core's batch row, cast to f32 -- exact for ids < 2^24), hyper/avoid
    [NSET] f32.  Output: out [slices, q, k] f32.
    """
    assert k % P == 0 and q % P == 0
    F = k // P  # ids per partition when k ids are spread over P partitions

    nc = bacc.Bacc("TRN2", target_bir_lowering=False, debug=False)
    attn = nc.dram_tensor("attn", [slices, q, k], F32, kind="ExternalInput").ap()
    noise = nc.dram_tensor("noise", [slices, q, k], F32, kind="ExternalInput").ap()
    ids = nc.dram_tensor("ids", [k], F32, kind="ExternalInput").ap()
    hyper = nc.dram_tensor("hyper", [NSET], F32, kind="ExternalInput").ap()
    avoid = nc.dram_tensor("avoid", [NSET], F32, kind="ExternalInput").ap()
    out = nc.dram_tensor("out", [slices, q, k], F32, kind="ExternalOutput").ap()
    scratch = nc.dram_tensor("scale_scratch", [k], F32).ap()

    with tile.TileContext(nc) as tc:
        with (
            tc.tile_pool(name="setup", bufs=1) as setup_pool,
            tc.tile_pool(name="scale", bufs=1) as scale_pool,
            tc.tile_pool(name="attn", bufs=bufs) as attn_pool,
            tc.tile_pool(name="noise", bufs=bufs) as noise_pool,
            tc.tile_pool(name="stats", bufs=2 * bufs) as stats_pool,
        ):
            # ---- one-time: scale row --------------------------------------
            # ids laid out [P, F] (id index = p*F + f); sets broadcast [P, 64]
            ids_sb = setup_pool.tile([P, F], F32, tag="ids")
            nc.sync.dma_start(ids_sb[:], ids.rearrange("(p f) -> p f", p=P))
            hyper_sb = setup_pool.tile([P, NSET], F32, tag="hyp")
            nc.sync.dma_start(
                hyper_sb[:], hyper.unsqueeze(0).to_broadcast((P, NSET))
            )
            avoid_sb = setup_pool.tile([P, NSET], F32, tag="avd")
            nc.sync.dma_start(
                avoid_sb[:], avoid.unsqueeze(0).to_broadcast((P, NSET))
            )

            # membership: eq[p, f, j] = (ids[p, f] == set[j]); reduce over j
            ids_b = ids_sb[:].unsqueeze(2).to_broadcast((P, F, NSET))
            eq = setup_pool.tile([P, F, NSET], F32, tag="eq")
            hmem = setup_pool.tile([P, F], F32, tag="hmem")
            nc.vector.tensor_tensor(
                eq[:], ids_b, hyper_sb[:].unsqueeze(1).to_broadcast((P, F, NSET)),
                op=OP.is_equal,
            )
            nc.vector.reduce_max(hmem[:], eq[:], axis=mybir.AxisListType.X)
            eq2 = setup_pool.tile([P, F, NSET], F32, tag="eq2")
            amem = setup_pool.tile([P, F], F32, tag="amem")
            nc.vector.tensor_tensor(
                eq2[:], ids_b, avoid_sb[:].unsqueeze(1).to_broadcast((P, F, NSET)),
                op=OP.is_equal,
            )
            nc.vector.reduce_max(amem[:], eq2[:], axis=mybir.AxisListType.X)

            # scale = (1 + 0.18*h) * (1 - 0.001*a)
            nc.vector.tensor_scalar(
                hmem[:], hmem[:], HYPER_DELTA, 1.0, OP.mult, OP.add
            )
            nc.vector.tensor_scalar(
                amem[:], amem[:], AVOID_DELTA, 1.0, OP.mult, OP.add
            )
            nc.vector.tensor_tensor(hmem[:], hmem[:], amem[:], op=OP.mult)

            # bounce through DRAM to broadcast the scale row to all partitions
            nc.sync.dma_start(scratch.rearrange("(p f) -> p f", p=P), hmem[:])
            scale_bc = scale_pool.tile([P, k], F32, tag="scale_bc")
            nc.sync.dma_start(
                scale_bc[:], scratch.unsqueeze(0).to_broadcast((P, k))
            )

            # ---- main loop: softmax((attn + 0.1*noise) * scale) over k ----
            # Values are ~N(0, 1.18) so exp never overflows in f32; skip the
            # max-subtraction pass (matches jax softmax to ~1e-7 rel).
            # qb query-blocks of 128 rows per tile: tiles are [P, qb, k]
            # (qb*k free elements), DMAs move qb MB at once.  Row r of
            # query-block g lives at tile[:, g, :] and softmax reduces per
            # (row, g) over k, so exp/mul run per-g on sub-APs.
            scale_bc3 = scale_bc[:].unsqueeze(1).to_broadcast((P, qb, k))
            store = getattr(nc, store_eng)

            def main_body():
                for s in range(slices):
                    for j in range(q // (P * qb)):
                        rows = slice(j * P * qb, (j + 1) * P * qb)
                        # [qb*P, k] DRAM region viewed as [P, qb, k]
                        a_src = attn[s, rows, :].rearrange(
                            "(g p) k -> p g k", p=P
                        )
                        n_src = noise[s, rows, :].rearrange(
                            "(g p) k -> p g k", p=P
                        )
                        o_dst = out[s, rows, :].rearrange(
                            "(g p) k -> p g k", p=P
                        )
                        a_t = attn_pool.tile([P, qb, k], F32, tag="a")
                        nc.sync.dma_start(a_t[:], a_src)
                        n_t = noise_pool.tile([P, qb, k], F32, tag="n")
                        nc.sync.dma_start(n_t[:], n_src)

                        if dma_only:  # bench-only: pure-DMA floor
                            store.dma_start(o_dst, a_t[:])
                            continue

                        # n = (noise * 0.1) + attn
                        nc.vector.scalar_tensor_tensor(
                            n_t[:], n_t[:], DISTRACTION_LEVEL, a_t[:],
                            op0=OP.mult, op1=OP.add,
                        )
                        # n *= scale[k]
                        nc.vector.tensor_tensor(
                            n_t[:], n_t[:], scale_bc3, op=OP.mult
                        )
                        # a = exp(n); ssum = rowsum(exp(n)) per query-block
                        ssum = stats_pool.tile([P, qb], F32, tag="ssum")
                        for g in range(qb):
                            nc.scalar.activation(
                                a_t[:, g, :], n_t[:, g, :], AFT.Exp,
                                accum_out=ssum[:, g : g + 1],
                            )
                        rcp = stats_pool.tile([P, qb], F32, tag="rcp")
                        nc.vector.reciprocal(rcp[:], ssum[:])
                        # n = a * (1/ssum)
                        for g in range(qb):
                            nc.scalar.mul(
                                n_t[:, g, :], a_t[:, g, :], rcp[:, g : g + 1]
                            )
                        store.dma_start(o_dst, n_t[:])

            if reps == 1:
                main_body()
            else:
                # benchmarking only: repeat the identical body on a HW loop
                with tc.For_i(0, reps, 1):
                    main_body()

    nc.compile()
    return nc


_NC_CACHE = {}

# winning variant (HW-measured): 2 query-blocks per DMA (2 MiB transfers),
# triple-buffered pools, loads+stores on the SP HWDGE queues
BUILD_KW = dict(qb=2, bufs=3, store_eng="sync")


def _get_nc(reps=1):
    key = (SLICES_PER_CORE, Q, K, reps)
    if key not in _NC_CACHE:
        _NC_CACHE[key] = build_nc(reps=reps, **BUILD_KW)
    return _NC_CACHE[key]


def _shard(attn_weights, noise, input_ids, hyperfocus_ids, avoid_ids):
    attn_flat = np.ascontiguousarray(attn_weights, dtype=np.float32).reshape(
        B * H, Q, K
    )
    noise_flat = np.ascontiguousarray(noise, dtype=np.float32).reshape(B * H, Q, K)
    hyper_f = np.asarray(hyperfocus_ids).astype(np.float32)
    avoid_f = np.asarray(avoid_ids).astype(np.float32)
    ids_f = np.asarray(input_ids).astype(np.float32)  # [B, K]

    in_maps = []
    for c in range(N_CORES):
        lo = c * SLICES_PER_CORE
        b = lo // H
        in_maps.append(
            {
                "attn": attn_flat[lo : lo + SLICES_PER_CORE],
                "noise": noise_flat[lo : lo + SLICES_PER_CORE],
                "ids": ids_f[b],
                "hyper": hyper_f,
                "avoid": avoid_f,
            }
        )
    return in_maps


def run_sharded(in_maps, trace=False, **kwargs):
    nc = _get_nc()
    return run_bass_kernel_spmd(
        nc, in_maps, core_ids=list(range(N_CORES)), trace=trace, **kwargs
    )


def kernel(attn_weights, noise, input_ids, hyperfocus_ids, avoid_ids):
    in_maps = _shard(attn_weights, noise, input_ids, hyperfocus_ids, avoid_ids)
    res = run_sharded(in_maps)
    parts = [res.results[c]["out"] for c in range(N_CORES)]
    full = np.concatenate(parts, axis=0).reshape(B, H, Q, K)
    return full
